# revision 24
# baseline (speedup 1.0000x reference)
"""Trainium2 Bass kernel for nn_MultiHeadAttention (B=2, T=2048, D=1024, H=16).

Sharding: 8 cores; core c owns head pair (2c, 2c+1) = output-channel slice
[c*128, (c+1)*128) of Wq/Wk/Wv columns and Wo rows (tensor parallel), both
batches. Host pre-transposes x and weight slices; each core computes a
partial output projection over its 128 ctx channels; host sums the 8
partials (replaces the all-reduce) and adds bo.

Per-core dataflow (all matmuls float32r, moving N=512):
  QT/KT[e,t] projections (xT moving), VT projection + PE-transpose to V
  natural [t,e] with a fused ones-column for the softmax denominator;
  per (batch, 1024-wide q-pair): scoresT[k,q] = KT.T @ QT row-tiled 2 heads
  into 2-bank PSUM, exp on ACT over [128,1024] (scale=1/8 fused), ctx
  accumulation ctxU_aug[65,1024] = [V|1].T @ escT over 16 k-tiles; 1/s via
  DVE reciprocal + PE outer-product broadcast; out-proj partial [t,e] =
  ctxT.T @ WoT_slice streamed to DRAM.
"""

import numpy as np

P = 128
D = 1024
BT = 4096
T = 2048
NB = 2
DC = 8    # D chunks of 128
TCH = 8   # 512-wide t-chunks over BT
KT = 16   # 128-wide k-tiles per batch
QC = 4    # 512-wide q-chunks per batch
NCORES = 8
DK = 64

_CACHE = {}


def _build(reps=1):
    import concourse.bass as bass
    import concourse.tile as tile
    from concourse import bacc, mybir
    from concourse.masks import make_identity

    f32 = mybir.dt.float32
    f32r = mybir.dt.float32r
    bf16 = mybir.dt.bfloat16
    Exp = mybir.ActivationFunctionType.Exp
    ds = bass.ds

    nc = bacc.Bacc("TRN2", target_bir_lowering=False, debug=False)

    xt = nc.dram_tensor("xt", [D, BT], f32r, kind="ExternalInput").ap()
    wq = nc.dram_tensor("wq", [D, P], f32r, kind="ExternalInput").ap()
    wk = nc.dram_tensor("wk", [D, P], f32r, kind="ExternalInput").ap()
    wv = nc.dram_tensor("wv", [D, P], f32r, kind="ExternalInput").ap()
    wo = nc.dram_tensor("wo", [P, D], f32r, kind="ExternalInput").ap()
    bqd = nc.dram_tensor("bq", [P, 1], f32, kind="ExternalInput").ap()
    bkd = nc.dram_tensor("bk", [P, 1], f32, kind="ExternalInput").ap()
    bvd = nc.dram_tensor("bv", [P, 1], f32, kind="ExternalInput").ap()
    out = nc.dram_tensor("out", [BT, D], f32, kind="ExternalOutput").ap()

    with tile.TileContext(nc) as tc:
        with (
            tc.tile_pool(name="const", bufs=1) as constp,
            tc.tile_pool(name="xtp", bufs=2) as xtp,
            tc.tile_pool(name="qkv", bufs=1) as qkvp,
            tc.tile_pool(name="vts", bufs=2) as vtsp,
            tc.tile_pool(name="esc", bufs=6) as escp,
            tc.tile_pool(name="ctx", bufs=2) as ctxp,
            tc.tile_pool(name="small", bufs=2) as smallp,
            tc.tile_pool(name="bsb", bufs=2) as bsbp,
            tc.tile_pool(name="psS", bufs=3, space="PSUM") as psS,
            tc.tile_pool(name="psC", bufs=2, space="PSUM") as psC,
        ):
            # ---- constants ----
            wq_sb = constp.tile([P, DC, P], f32r, tag="wq")
            wk_sb = constp.tile([P, DC, P], f32r, tag="wk")
            wv_sb = constp.tile([P, DC, P], f32r, tag="wv")
            bq_sb = constp.tile([P, 1], f32, tag="bq")
            nc.sync.dma_start(bq_sb, bqd)
            bk_sb = constp.tile([P, 1], f32, tag="bk")
            nc.sync.dma_start(bk_sb, bkd)
            bv_sb = constp.tile([P, 1], f32, tag="bv")
            nc.sync.dma_start(bv_sb, bvd)
            ident_f = constp.tile([P, P], f32, tag="identf")
            make_identity(nc, ident_f)
            ident = constp.tile([P, P], f32r, tag="ident")
            nc.vector.tensor_copy(ident, ident_f)
            ones_f32 = constp.tile([P, DK], f32, tag="ones_f32")
            nc.vector.memset(ones_f32, 1.0)
            ones_t = constp.tile([P, DK], f32r, tag="ones")
            nc.vector.tensor_copy(ones_t, ones_f32)
            wo_sb = constp.tile([P, D], f32r, tag="wo")

            # ---- per-batch persistent tiles ----
            qt_sb = [
                qkvp.tile([P, T], bf16, tag=f"qt{b}", name=f"qt{b}")
                for b in range(NB)
            ]
            kt_sb = [
                qkvp.tile([P, T], bf16, tag=f"kt{b}", name=f"kt{b}")
                for b in range(NB)
            ]
            # V natural per batch per head, 65-wide blocks: [V(64) | ones]
            va_sb = [
                qkvp.tile([P, KT * 65], bf16, tag=f"va{b}", name=f"va{b}")
                for b in range(NB)
            ]
            vb_sb = [
                qkvp.tile([P, KT * 65], bf16, tag=f"vb{b}", name=f"vb{b}")
                for b in range(NB)
            ]
            ones_col = ones_f32[:, 0:KT].rearrange("p (k one) -> p k one", one=1)
            for b in range(NB):
                nc.vector.tensor_copy(
                    va_sb[b].rearrange("p (k c) -> p k c", c=65)[:, :, 64:65],
                    ones_col,
                )
                nc.vector.tensor_copy(
                    vb_sb[b].rearrange("p (k c) -> p k c", c=65)[:, :, 64:65],
                    ones_col,
                )

            xt_r = xt.rearrange("(c p) t -> p c t", p=P)

            for _rep in range(reps):

                def proj_chunk(tch, xtile=None):
                    b = tch // 4
                    tloc = (tch % 4) * 512  # within-batch t offset
                    if xtile is None:
                        xtile = xtp.tile([P, DC, 512], f32r, tag="xt", name="xtile")
                        nc.sync.dma_start(xtile, xt_r[:, :, ds(tch * 512, 512)])

                    psq = psS.tile([P, 512], f32, tag="sc", name="psq")
                    for c in range(DC):
                        nc.tensor.matmul(
                            psq, wq_sb[:, c], xtile[:, c],
                            start=(c == 0), stop=(c == DC - 1),
                        )
                    nc.vector.tensor_scalar_add(
                        qt_sb[b][:, ds(tloc, 512)], psq, bq_sb
                    )

                    psk = psS.tile([P, 512], f32, tag="sc", name="psk")
                    for c in range(DC):
                        nc.tensor.matmul(
                            psk, wk_sb[:, c], xtile[:, c],
                            start=(c == 0), stop=(c == DC - 1),
                        )
                    nc.vector.tensor_scalar_add(
                        kt_sb[b][:, ds(tloc, 512)], psk, bk_sb
                    )

                    psv = psS.tile([P, 512], f32, tag="sc", name="psv")
                    for c in range(DC):
                        nc.tensor.matmul(
                            psv, wv_sb[:, c], xtile[:, c],
                            start=(c == 0), stop=(c == DC - 1),
                        )
                    vts = vtsp.tile([P, 512], f32r, tag="vts", name="vts")
                    nc.vector.tensor_scalar_add(vts, psv, bv_sb)
                    # transpose VT [e,512] -> V natural [t,e] in 128-tiles
                    for tt in range(4):
                        ktile = (tch % 4) * 4 + tt  # k-tile index within batch
                        pvt = psS.tile([P, P], f32r, tag="sc", name="pvt")
                        nc.tensor.transpose(pvt, vts[:, ds(tt * P, P)], ident)
                        nc.vector.tensor_copy(
                            va_sb[b][:, ds(ktile * 65, DK)], pvt[:, 0:DK]
                        )
                        nc.vector.tensor_copy(
                            vb_sb[b][:, ds(ktile * 65, DK)], pvt[:, DK:P]
                        )

                # pipelined finalize: stage1 (bcast+normalize) and stage2
                # (out-projection) of the previous chunk are emitted inside
                # the current chunk's kt loop to keep the PE stream dense.
                def fin_stage1(st):
                    b, qch, ua, ub = st
                    rf = smallp.tile([P, 1024], f32, tag="recipf", name="rf")
                    nc.vector.reciprocal(rf[64:65, 0:512], ua[64:65, :])
                    nc.vector.reciprocal(rf[64:65, 512:1024], ub[64:65, :])
                    rr = smallp.tile([P, 1024], f32r, tag="recip", name="rr")
                    nc.vector.tensor_copy(rr[64:65, :], rf[64:65, :])
                    bc = psS.tile([P, 1024], f32, tag="sc", name="bc")
                    nc.tensor.matmul(
                        bc[0:DK, 0:512], ones_t[64:65, :], rr[64:65, 0:512],
                        start=True, stop=True, tile_position=(64, 0),
                    )
                    nc.tensor.matmul(
                        bc[0:DK, 512:1024], ones_t[64:65, :],
                        rr[64:65, 512:1024],
                        start=True, stop=True, tile_position=(64, 0),
                    )
                    bc_sb = bsbp.tile([DK, 1024], f32, tag="bcs", name="bc_sb")
                    nc.vector.tensor_copy(bc_sb, bc[0:DK, :])
                    ctq = ctxp.tile([P, 512], f32r, tag="ctq", name="ctq")
                    nc.vector.tensor_mul(
                        ctq[0:DK, :], ua[0:DK, :], bc_sb[:, 0:512]
                    )
                    tmpb = bsbp.tile([DK, 512], f32r, tag="tmpb", name="tmpb")
                    nc.vector.tensor_mul(tmpb, ub[0:DK, :], bc_sb[:, 512:1024])
                    nc.sync.dma_start(ctq[DK:P, :], tmpb)
                    return ctq

                def fin_stage2(st, ctq, tts):
                    b, qch, ua, ub = st
                    q0 = qch * 512
                    for tt in tts:
                        po = psS.tile([P, 1024], f32, tag="sc", name="po")
                        nc.tensor.matmul(
                            po[:, 0:512],
                            ctq[:, ds(tt * P, P)], wo_sb[:, 0:512],
                            start=True, stop=True,
                        )
                        nc.tensor.matmul(
                            po[:, 512:1024],
                            ctq[:, ds(tt * P, P)], wo_sb[:, 512:1024],
                            start=True, stop=True,
                        )
                        po_sb = escp.tile([P, 1024], f32, tag="posb", name="po_sb", bufs=3)
                        nc.vector.tensor_copy(po_sb, po)
                        r0 = b * T + q0 + tt * P
                        nc.sync.dma_start(out[r0 : r0 + P, :], po_sb)

                pending = {"st": None, "ctq": None}

                def drain_pending(kt):
                    # interleave previous chunk's finalize into this kt loop;
                    # stage1 late enough that the DVE reciprocal chain has
                    # completed (it starts right after the previous chunk)
                    if pending["st"] is None:
                        return
                    if kt == 8:
                        pending["ctq"] = fin_stage1(pending["st"])
                    elif kt == 12:
                        fin_stage2(pending["st"], pending["ctq"], (0, 1))
                    elif kt == 15:
                        fin_stage2(pending["st"], pending["ctq"], (2, 3))
                        pending["st"] = None

                def attn_chunk(b, qch):
                    q0 = qch * 512
                    cxa = psC.tile([65, 512], f32, tag="cx", name="cxa")
                    cxb = psC.tile([65, 512], f32, tag="cx", name="cxb")
                    for kt in range(KT):
                        sc = psS.tile([P, 1024], f32, tag="sc", name="sc")
                        nc.tensor.matmul(
                            sc[:, 0:512],
                            kt_sb[b][0:DK, ds(kt * P, P)],
                            qt_sb[b][0:DK, ds(q0, 512)],
                            start=True, stop=True,
                        )
                        nc.tensor.matmul(
                            sc[:, 512:1024],
                            kt_sb[b][DK:P, ds(kt * P, P)],
                            qt_sb[b][DK:P, ds(q0, 512)],
                            start=True, stop=True,
                            tile_position=(64, 0),
                        )
                        esc = escp.tile([P, 1024], bf16, tag="esc", name="esc")
                        nc.scalar.activation(esc, sc, Exp, scale=0.125)
                        nc.tensor.matmul(
                            cxa,
                            va_sb[b][:, ds(kt * 65, 65)],
                            esc[:, 0:512],
                            start=(kt == 0), stop=(kt == KT - 1),
                        )
                        nc.tensor.matmul(
                            cxb,
                            vb_sb[b][:, ds(kt * 65, 65)],
                            esc[:, 512:1024],
                            start=(kt == 0), stop=(kt == KT - 1),
                        )
                        drain_pending(kt)
                    ua = bsbp.tile([65, 512], f32, tag="ua", name="ua")
                    nc.vector.tensor_copy(ua, cxa)
                    ub = bsbp.tile([65, 512], f32, tag="ub", name="ub")
                    nc.vector.tensor_copy(ub, cxb)
                    return (b, qch, ua, ub)

                if _rep == 0:
                    xtile0 = xtp.tile([P, DC, 512], f32r, tag="xt", name="xtile")
                    nc.sync.dma_start(xtile0, xt_r[:, :, ds(0, 512)])
                    nc.sync.dma_start(
                        wq_sb, wq.rearrange("(c p) e -> p c e", p=P)
                    )
                    nc.sync.dma_start(
                        wk_sb, wk.rearrange("(c p) e -> p c e", p=P)
                    )
                    nc.sync.dma_start(
                        wv_sb, wv.rearrange("(c p) e -> p c e", p=P)
                    )
                    proj_chunk(0, xtile0)
                else:
                    proj_chunk(0)
                # wo needed only at first finalize; keep it off the critical
                # startup DMA path
                if _rep == 0:
                    nc.sync.dma_start(wo_sb, wo)
                for tch in range(1, TCH):
                    proj_chunk(tch)
                for b in range(NB):
                    for qch in range(4):
                        pending["st"] = attn_chunk(b, qch)
                ctq = fin_stage1(pending["st"])
                fin_stage2(pending["st"], ctq, (0, 1, 2, 3))

    nc.compile()
    return nc


def _get_nc(reps=1):
    key = f"nc{reps}"
    if key not in _CACHE:
        _CACHE[key] = _build(reps)
    return _CACHE[key]


def kernel(x, Wq, bq, Wk, bk, Wv, bv, Wo, bo):
    from concourse.bass_utils import run_bass_kernel_spmd

    x = np.asarray(x, dtype=np.float32)
    Wq = np.asarray(Wq, dtype=np.float32)
    Wk = np.asarray(Wk, dtype=np.float32)
    Wv = np.asarray(Wv, dtype=np.float32)
    Wo = np.asarray(Wo, dtype=np.float32)
    bq = np.asarray(bq, dtype=np.float32)
    bk = np.asarray(bk, dtype=np.float32)
    bv = np.asarray(bv, dtype=np.float32)
    bo = np.asarray(bo, dtype=np.float32)

    B, Tl, Dl = x.shape
    xt = np.ascontiguousarray(x.reshape(B * Tl, Dl).T)

    in_maps = []
    for c in range(NCORES):
        sl = slice(c * P, (c + 1) * P)
        in_maps.append(
            {
                "xt": xt,
                "wq": np.ascontiguousarray(Wq[sl, :].T),
                "wk": np.ascontiguousarray(Wk[sl, :].T),
                "wv": np.ascontiguousarray(Wv[sl, :].T),
                "wo": np.ascontiguousarray(Wo[:, sl].T),
                "bq": np.ascontiguousarray(bq[sl].reshape(P, 1)),
                "bk": np.ascontiguousarray(bk[sl].reshape(P, 1)),
                "bv": np.ascontiguousarray(bv[sl].reshape(P, 1)),
            }
        )

    nc = _get_nc()
    _CACHE["in_maps"] = in_maps
    res = run_bass_kernel_spmd(nc, in_maps, core_ids=list(range(NCORES)))
    acc = res.results[0]["out"].astype(np.float32)
    for c in range(1, NCORES):
        acc = acc + res.results[c]["out"]
    acc = acc + bo[None, :]
    return acc.reshape(B, Tl, Dl).astype(np.float32)
